# revision 42
# baseline (speedup 1.0000x reference)
"""Single-head attention kernel for Trainium2, 8 NeuronCores SPMD (v2).

Problem: x[4,4096,1024] @ {Wq,Wk,Wv}[1024,128] -> q,k,v; softmax(q k^T/sqrt(128)) v.

Sharding: core c -> (batch b = c//2, query-half h = c%2).
Each core receives xT = x[b].T in bf16 with the 4096 columns permuted so
"my" 2048 query rows come first; it computes kT/vn for all 4096 keys, qT
for its 2048 queries, and emits outT [128, 2048] fp32.

Key design points vs the f32r two-phase version (181us -> ~128us measured):
  * Host-side pre-layout: x and W arrive in DRAM already in the SBUF tile
    layout, so every DMA is a contiguous 1:1 copy (128 descriptors, >=4KB
    runs). The strided rearranging DMAs of the old version generated ~11k
    descriptors and made the DMA engines the pacing unit (~177us) while
    compute took ~122us.
  * bf16 everywhere on the matmul paths: halves the x DMA (8MB), halves
    SBUF footprints, enables FWL fast weight loads, and gives 2x DVE on
    the exp-sum (denominator) adds. PSUM stays fp32.
  * ONE fused pass: a minimal prologue (kT block 0, qT blocks 0-1), then
    every remaining projection is a JIT job interleaved into the attention
    groups at explicit iteration positions ahead of its deadline, so PE
    work tracks ACT's exp load window by window and PE stays HAM-warm.
  * Denominator entirely off the PE: dacc (bf16, DVE) accumulates the exp
    tiles; two accumulating ones-matmuls per q-block do the partition sum
    at the end. The old version spent ~27us of PE streaming on it.
  * Bias algebra: bk shifts every score of a query by a constant, which
    softmax cancels -> dropped. bv folds into the vT evacuation (dv is
    the partition dim there; attn rows sum to 1). Only bq is applied on
    the qT evacuation.

On-chip layouts (SBUF [128 part x free]):
  kT,qT [d=128, seq] bf16; vn = natural-v chunks [128, S] bf16;
  u = exp tiles [128, 2 x 512*len(pair)] bf16 (two exps share one tile
  so the DVE denominator add runs once per pair of chunks);
  out_acc [128, SQ] fp32 accumulates PV partials evacuated from PSUM.

PSUM (8 banks exactly):
  pps 2x[128,512] f32  - projection accumulators + v-transpose outputs
  sps 2x[128,1024] f32 - score tiles (2 q-blocks wide), 1024-wide exp
  po  1x[128,1024] f32 - PV accumulator for a q-block pair over an
                         8/16-chunk range, then DVE-added into out_acc
                         (also recycled for the dps/bps epilogue tiles)
"""

import math

import numpy as np

import concourse.bacc as bacc
import concourse.bass as bass
import concourse.mybir as mybir
from concourse.bass import ts
from concourse.masks import make_identity
from concourse.tile import TileContext

P = 128
D_MODEL = 1024
D_QK = 128
B = 4
S_FULL = 4096
N_CORES = 8
BLK = 512  # projection block (columns of xT)
BPG = 2  # blocks per group (projection/DMA granularity)

F32 = mybir.dt.float32
F32R = mybir.dt.float32r
BF16 = mybir.dt.bfloat16
AF = mybir.ActivationFunctionType

SM_SCALE = 1.0 / math.sqrt(D_QK)


def build_attention(nc: bass.Bass, S: int = S_FULL, SQ: int = S_FULL // 2, repeat: int = 1):
    """Emit the SPMD single-core program. S = #keys, SQ = #queries."""
    DC = D_MODEL // P  # 8 d_model chunks
    NBLK = S // BLK  # xT column blocks
    QNB = SQ // BLK  # query blocks
    assert NBLK % BPG == 0 and QNB <= NBLK
    NG = NBLK // BPG  # groups
    CPB = BLK // P  # k-chunks per block (4)
    CPG = CPB * BPG  # k-chunks per group (8)
    KC = S // P  # total k chunks
    # q-block pairs; pair p covers q-blocks (2p, 2p+1)
    pairs = [tuple(range(i, min(i + 2, QNB))) for i in range(0, QNB, 2)]
    # group after which pair p's qT columns exist
    avail = [max(pr) // BPG for pr in pairs]

    # All inputs are pre-arranged on the host into the exact SBUF layouts so
    # every DMA is a contiguous 1:1 copy with >=2KB runs per partition.
    # Strided descriptor-per-chunk DMAs were the kernel's real bottleneck:
    # ~11k descriptors made the DMA engines the pacing unit (~177us) while
    # compute finished in ~122us.
    xh = nc.dram_tensor("xh", [NBLK * P, DC * BLK], BF16, kind="ExternalInput").ap()
    wq = nc.dram_tensor("Wq", [P, DC * D_QK], BF16, kind="ExternalInput").ap()
    wk = nc.dram_tensor("Wk", [P, DC * D_QK], BF16, kind="ExternalInput").ap()
    wv = nc.dram_tensor("Wv", [P, DC * D_QK], BF16, kind="ExternalInput").ap()
    bqv = nc.dram_tensor("bqv", [2, D_QK], F32R, kind="ExternalInput").ap()
    outT = nc.dram_tensor("outT", [D_QK, SQ], F32, kind="ExternalOutput").ap()

    with TileContext(nc) as tc:
        lp = nc.allow_low_precision(reason="bf16 accumulate of positive exp values")
        lp.__enter__()
        if repeat > 1:
            loop_cm = tc.For_i(0, repeat, 1)
            loop_cm.__enter__()
        with (
            tc.tile_pool(name="persist", bufs=1) as pp,
            tc.tile_pool(name="xt_pool", bufs=NBLK) as xp,
            tc.tile_pool(name="u_pool", bufs=6) as up,
            tc.tile_pool(name="wkb", bufs=6) as wkb,
            tc.tile_pool(name="pps", bufs=2, space="PSUM") as pps,
            tc.tile_pool(name="sps", bufs=2, space="PSUM") as spsp,
            tc.tile_pool(name="po", bufs=1, space="PSUM") as pop,
        ):
            # --- tiles ---
            w_sb = {
                nm: pp.tile([P, DC * D_QK], BF16, tag=f"w{nm}", name=f"w{nm}_sb")
                for nm in ("q", "k", "v")
            }
            w_dram = {"q": wq, "k": wk, "v": wv}
            bqv_row = pp.tile([2, D_QK], F32R, tag="bqv_row")
            bq_sb = pp.tile([P, 1], F32, tag="bq_sb")
            bv4 = pp.tile([P, BLK], F32, tag="bv4")  # bv bcast, 4x tiled
            # ident first on Pool: the PE warmup chain waits only on it
            ident = pp.tile([P, P], F32, tag="ident")
            make_identity(nc, ident)
            # selector [2,128]: row0=0, row1=1 -> matmul picks bqv_row's bv
            # row and replicates it across all 128 output partitions.
            # (memset on F32R fails the neuronxcc ISA check; memset F32 and
            # DVE-copy into the F32R tile, like ident_r)
            sel01_f = pp.tile([2, P], F32, tag="sel01f")
            nc.gpsimd.memset(sel01_f, 1.0)
            nc.gpsimd.memset(sel01_f[0:1, :], 0.0)
            sel01 = pp.tile([2, P], F32R, tag="sel01")
            nc.vector.tensor_copy(out=sel01, in_=sel01_f)
            ident_r = pp.tile([P, P], F32R, tag="ident_r")
            nc.vector.tensor_copy(out=ident_r, in_=ident)
            # all-ones [128,128] stationary: partition-sums land replicated
            # across all 128 output partitions (feeds tensor_mul directly)
            ones128 = pp.tile([P, P], BF16, tag="ones128")
            nc.gpsimd.memset(ones128, 1.0)
            ones_row = pp.tile([1, P], F32, tag="ones_row")
            nc.gpsimd.memset(ones_row, 1.0)
            ones_row_r = pp.tile([1, P], F32R, tag="ones_row_r")  # lhsT for bcast
            nc.vector.tensor_copy(out=ones_row_r, in_=ones_row)

            # PE warmup: junk matmuls reading only Pool-memset data. Two
            # jobs: (a) the head event-collector (compile merges the first
            # batch's waits into one EventSemaphore) then waits only on the
            # ~0.4us memsets instead of the xt0 DMA quarters (~3.5us); (b)
            # ~2.5us of continuous PE busy ramps the pstate toward 2.4GHz
            # before the first projection (which otherwise runs at 0.65GHz).
            def pe_warmup(n):
                scr = pps.tile([P, BLK], F32, tag="pps", name="warm_scr")
                for i in range(n):
                    nc.tensor.matmul(
                        scr[0:1, 0:P], ident[:, 0:1], ident, start=True, stop=True
                    )

            kT = pp.tile([P, S], BF16, tag="kT")
            vn = pp.tile([P, S], BF16, tag="vn")
            qT = pp.tile([P, SQ], BF16, tag="qT")
            out_acc = pp.tile([P, SQ], F32, tag="out_acc")
            daccs = [
                pp.tile([P, 2 * BLK * len(pr)], BF16, tag=f"dacc{p}", name=f"dacc{p}")
                for p, pr in enumerate(pairs)
            ]

            def dma_w(nm):
                nc.sync.dma_start(out=w_sb[nm], in_=w_dram[nm])

            # Wait-absorbers: LDWEIGHTS can encode only one sync wait; tiny
            # PE matmuls read each DMA'd / off-engine-produced tile so real
            # matmuls carry at most one un-observed semaphore (the rest are
            # split into event semaphores by nc.compile(), which is slow).
            def pe_absorb_simple(ap):
                # wait-absorber: a tiny matmul reads col 0 of the tile so PE
                # observes its producer semaphore; real matmuls then carry at
                # most one un-observed wait (the LDWEIGHTS wait-slot limit)
                scr = pps.tile([P, BLK], F32, tag="pps", name="absorb_scr")
                a = ap[:, 0:1]
                if a.dtype == F32R:
                    a = a.bitcast(F32)
                nc.tensor.matmul(scr[0:1, 0:1], a, a, start=True, stop=True)

            # --- DMA issue order (one queue set, ~in-order): Wk and block 0
            # first so the first projection starts ASAP, then the rest.
            xts = []
            hc = DC // 2

            def dma_x(b):
                xt = xp.tile([P, DC * BLK], BF16, tag="xt", name=f"xt_{b}")
                src = xh[b * P : (b + 1) * P, :]
                if b < 2:
                    # startup-critical blocks: quarters so the first
                    # projection matmuls unblock as data trickles in
                    nparts = 4
                else:
                    # halves: proj half-jobs consume d-chunks 0-3 then 4-7,
                    # so half-block arrivals unblock them sooner
                    nparts = 2
                qw = DC * BLK // nparts
                for i in range(nparts):
                    nc.sync.dma_start(
                        out=xt[:, i * qw : (i + 1) * qw],
                        in_=src[:, i * qw : (i + 1) * qw],
                    )
                xts.append(xt.rearrange("p (c s) -> p c s", s=BLK))

            # SP-queue order tuned so the attention-gating transfers (wk,
            # xt0, wq, wv, xt1) land first; bqv rides the otherwise-idle ACT
            # hwdge queue so its 500ns never delays the big transfers.
            nc.scalar.dma_start(out=bqv_row, in_=bqv)
            dma_w("k")
            dma_x(0)  # quarters interleave with wq/wv below via queue order
            dma_w("q")
            dma_w("v")
            dma_x(1)
            for b in range(2, NBLK):
                dma_x(b)
            # attention-gating DMA chain: wk+xt0+wq+wv+xt1 = 2.77MB ~ 8.9us;
            # the prologue (k0,q0,v0,q1) overlaps it, so scores c0 can issue
            # ~1us after xt1 lands.

            absorbed = set()

            # Wait-absorber experiment: per-queue sems (DVE_49, DMAHWn_49,
            # ...) are monotone counters, so a single wait at the max value
            # covers every earlier producer on that queue; each matmul's
            # LDWEIGHTS + MATMUL can each carry one wait. Absorbs disabled
            # unless a consumer instruction really accumulates two
            # unobserved sems (re-enable via ABSORB if compile inserts
            # event-collectors and stalls appear).
            ABSORB = False

            def absorb_once(key, ap):
                if ABSORB and key not in absorbed:
                    pe_absorb_simple(ap)
                    absorbed.add(key)

            def emit_bias():
                # broadcast bq/bv rows ([2,128] dram) to [128,1] per-partition
                # scalars via one tiny transpose-by-matmul against I2.
                # Emitted AFTER proj_k(0): the bqv DMA rides the ACT queue
                # behind the 1.3us act-table load, so putting this at the PE
                # stream head stalled the whole engine ~3.7us.
                absorb_once("bqv", bqv_row)  # bias MM then waits only ident_r
                bias_ps = pop.tile([P, 2 * BLK], F32, tag="po", name="bias_ps")
                nc.tensor.matmul(
                    bias_ps[:, 0:2], bqv_row, ident_r[0:2, 0:2], start=True, stop=True
                )
                nc.vector.tensor_copy(out=bq_sb, in_=bias_ps[:, 0:1])

            def emit_bias_v():
                # bv broadcast to [128 part, 4x128] straight from bqv_row
                # (in SBUF ~2us): out[m, n] = sum_k sel01[k, m]*bqv_row[k, n]
                # = bv[n] for every partition m. No extra DMA — a host-tiled
                # bv row riding the ACT queue got served at ~5us by the
                # shared DMA device and stalled the whole in-order PE head.
                bcast_ps = pop.tile([P, 2 * BLK], F32, tag="po", name="bcast_ps")
                for i in range(CPB):
                    nc.tensor.matmul(
                        bcast_ps[:, BLK + i * P : BLK + (i + 1) * P],
                        sel01, bqv_row, start=True, stop=True,
                    )
                nc.vector.tensor_copy(out=bv4, in_=bcast_ps[:, BLK : 2 * BLK])

            proj_open = {}  # (kind, b) -> open PSUM accumulator (half-jobs)

            def proj_k(b, absorb=True, half=None):
                # half=0: first 4 contraction chunks; half=1: rest + evac.
                # Splitting spreads PE filler smoothly between att iterations.
                if half == 1:
                    kps = proj_open.pop(("k", b))
                    cr = range(DC // 2, DC)
                else:
                    absorb_once("wk", w_sb["k"])
                    kps = pps.tile([P, BLK], F32, tag="pps", name=f"kps_{b}")
                    cr = range(DC) if half is None else range(DC // 2)
                for c in cr:
                    nc.tensor.matmul(
                        kps,
                        w_sb["k"][:, ts(c, D_QK)],
                        xts[b][:, c],
                        start=(c == 0),
                        stop=(c == DC - 1),
                    )
                if half == 0:
                    proj_open[("k", b)] = kps
                    return
                nc.vector.tensor_copy(out=kT[:, ts(b, BLK)], in_=kps)
                if absorb and ABSORB:
                    pe_absorb_simple(kT[:, ts(b, BLK)])

            def proj_v(b, absorb=True, half=None):
                # natural-layout v directly: per 128-key s-group j, accumulate
                # out[s, dv] over the 8 d-chunks with lhsT = xt slice (the
                # stationary) and rhs = Wv chunk. Same PE column count as the
                # transposed projection but WITHOUT the 4 PE transposes per
                # block, and bv rides the PSUM evacuation add (vs a separate
                # tensor_scalar pass on the old vT path).
                if half == 1:
                    vps = proj_open.pop(("v", b))
                    gr = range(CPB // 2, CPB)
                else:
                    vps = pps.tile([P, BLK], F32, tag="pps", name=f"vps_{b}")
                    gr = range(CPB) if half is None else range(CPB // 2)
                for j in gr:
                    for c in range(DC):
                        nc.tensor.matmul(
                            vps[:, ts(j, P)],
                            xts[b][:, c][:, ts(j, P)],
                            w_sb["v"][:, ts(c, D_QK)],
                            start=(c == 0),
                            stop=(c == DC - 1),
                        )
                if half == 0:
                    proj_open[("v", b)] = vps
                    return
                # attention rows sum to 1, so out = attn @ (v + bv): add the
                # broadcast bv row during the PSUM evacuation
                nc.vector.tensor_add(
                    out=vn[:, ts(b, BLK)], in0=vps, in1=bv4
                )
                if absorb and ABSORB:
                    pe_absorb_simple(vn[:, ts(b, BLK)])

            def proj_q(b, absorb=True, half=None):
                if half == 1:
                    qps = proj_open.pop(("q", b))
                    cr = range(DC // 2, DC)
                else:
                    absorb_once("wq", w_sb["q"])
                    qps = pps.tile([P, BLK], F32, tag="pps", name=f"qps_{b}")
                    cr = range(DC) if half is None else range(DC // 2)
                for c in cr:
                    nc.tensor.matmul(
                        qps,
                        w_sb["q"][:, ts(c, D_QK)],
                        xts[b][:, c],
                        start=(c == 0),
                        stop=(c == DC - 1),
                    )
                if half == 0:
                    proj_open[("q", b)] = qps
                    return
                nc.vector.tensor_scalar_add(qT[:, ts(b, BLK)], qps, bq_sb)
                if absorb and ABSORB:
                    pe_absorb_simple(qT[:, ts(b, BLK)])

            def proj_block(b):
                proj_k(b)
                proj_v(b)
                if b < QNB:
                    proj_q(b)

            dacc_started = [False] * len(pairs)
            done_units = [0] * len(pairs)
            total_units = [KC] * len(pairs)
            last_evac = [None]  # most recent po-evac destination (for absorb)

            def epilogue_pair(p, pr, final_us=None):
                """Per-qb: partition-sum dacc with an all-ones [128,128]
                stationary so the sums land REPLICATED across partitions
                (skips the old [1,BLK]->bcast matmul hop); the final chunk's
                exp tile (final_us) joins the PSUM accumulation directly so
                the last DVE dacc add leaves the critical tail."""
                w = BLK * len(pr)
                dacc = daccs[p]
                fin = wkb.tile([P, w], F32, tag="fin", name=f"fin_{p}")
                for j, qb in enumerate(pr):
                    # dps tiles come from the pps pool: the projections are
                    # long done, so no PSUM-bank WAR (po/sps banks carry the
                    # final range's PV/scores and would chain the epilogue
                    # behind the out_acc evacuation)
                    dps = pps.tile([P, BLK], F32, tag="pps", name=f"ep_{qb}")
                    # partition-sum of both chunk-parity halves of dacc
                    nc.tensor.matmul(
                        dps, ones128, dacc[:, ts(j, BLK)], start=True, stop=False
                    )
                    nc.tensor.matmul(
                        dps, ones128, dacc[:, w + j * BLK : w + (j + 1) * BLK],
                        start=False, stop=final_us is None,
                    )
                    if final_us is not None:
                        nc.tensor.matmul(
                            dps, ones128, final_us[:, ts(j, BLK)],
                            start=False, stop=True,
                        )
                    rec = wkb.tile([P, BLK], F32, tag="rec", name=f"rec_{qb}")
                    nc.vector.reciprocal(out=rec, in_=dps)
                    # normalize multiply on Pool (all-SBUF operands): runs
                    # parallel to DVE's evac/reciprocal chain in the tail
                    nc.gpsimd.tensor_mul(
                        out=fin[:, ts(j, BLK)], in0=out_acc[:, ts(qb, BLK)], in1=rec
                    )
                    nc.sync.dma_start(
                        out=outT[:, ts(qb, BLK)], in_=fin[:, ts(j, BLK)]
                    )

            def att_all(jobs_by_g):
                """Attention over all groups in one software-pipelined PE
                stream: scores(i+1) is emitted BEFORE PV(i), so PE always has
                the next score tile to chew while ACT computes exp(i).
                Without the skew, PV(i) (in-order PE) stalls a full exp
                latency (~1us) whenever projection filler runs out."""
                stream = []  # (g, it_in_group, p, pr, c, first, last)
                for g in range(NG):
                    pair_order = (
                        list(enumerate(pairs))[::-1]
                        if g == NG - 1
                        else list(enumerate(pairs))
                    )
                    it = 0
                    for p, pr in pair_order:
                        if g < avail[p]:
                            continue
                        lo = g * CPG if g > avail[p] else 0
                        rng = list(range(lo, (g + 1) * CPG))
                        for i, c in enumerate(rng):
                            stream.append(
                                (g, it, p, pr, c, i == 0, i == len(rng) - 1)
                            )
                            it += 1
                job_at = {}
                for g, jl in jobs_by_g.items():
                    for pos_i, job in jl:
                        job_at.setdefault((g, pos_i), []).append(job)

                def emit_pv(ent):
                    p, pr, c, first, last, po, us, finishing = ent
                    w = BLK * len(pr)
                    for j, qb in enumerate(pr):
                        nc.tensor.matmul(
                            po[:, ts(j, BLK)],
                            vn[:, ts(c, P)],
                            us[:, ts(j, BLK)],
                            start=first,
                            stop=last,
                        )
                    if last:
                        dst = out_acc[:, pr[0] * BLK : (pr[-1] + 1) * BLK]
                        if done_units[p] == 0:
                            nc.vector.tensor_copy(out=dst, in_=po[:, 0:w])
                        elif finishing and len(pr) > 1:
                            # split the final evac per qb so qb0's normalize
                            # multiply starts after only half the add
                            # (GPSIMD can't read PSUM on real HW, so these
                            # stay on DVE)
                            for j, qb in enumerate(pr):
                                nc.vector.tensor_add(
                                    out=out_acc[:, ts(qb, BLK)],
                                    in0=out_acc[:, ts(qb, BLK)],
                                    in1=po[:, ts(j, BLK)],
                                )
                        else:
                            nc.vector.tensor_add(out=dst, in0=dst, in1=po[:, 0:w])
                        done_units[p] += done_inc[p]
                        if done_units[p] == total_units[p]:
                            epilogue_pair(p, pr, final_us=us if finishing else None)

                pos = {}  # p -> [po tile, u tile]
                done_inc = {}
                pending = None  # deferred PV of the previous iteration
                for g, it, p, pr, c, first, last in stream:
                    for job in job_at.get((g, it), []):
                        job()
                    w = BLK * len(pr)
                    if first:
                        pos[p] = [
                            pop.tile([P, 2 * BLK], F32, tag="po", name=f"po_{g}_{p}"),
                            pos.get(p, [None, None])[1],
                        ]
                        done_inc[p] = CPG if g > avail[p] else (g + 1) * CPG
                    po = pos[p][0]
                    sps = spsp.tile([P, 1024], F32, tag="sps", name=f"s_{g}_{p}_{c}")
                    for j, qb in enumerate(pr):
                        nc.tensor.matmul(
                            sps[:, ts(j, BLK)],
                            kT[:, ts(c, P)],
                            qT[:, ts(qb, BLK)],
                            start=True,
                            stop=True,
                        )
                    par = c % 2
                    if par == 0:
                        pos[p][1] = up.tile(
                            [P, 2 * w], BF16, tag="u", name=f"u_{g}_{p}_{c}"
                        )
                    u = pos[p][1]
                    us = u[:, par * w : par * w + w]
                    # does this chunk end the pair's final range?
                    finishing = (
                        last and done_units[p] + done_inc[p] == total_units[p]
                    )
                    if finishing and len(pr) > 1:
                        # split the final exp per qb: qb0's epilogue chain
                        # (dps matmul on its half) starts ~0.5us earlier
                        nc.scalar.activation(
                            us[:, 0:BLK], sps[:, 0:BLK], AF.Exp, scale=SM_SCALE
                        )
                        nc.scalar.activation(
                            us[:, BLK:w], sps[:, BLK:w], AF.Exp, scale=SM_SCALE
                        )
                    else:
                        nc.scalar.activation(us, sps[:, 0:w], AF.Exp, scale=SM_SCALE)
                    if finishing:
                        # final chunk's u joins the dps PSUM accumulation on
                        # PE (epilogue final_us) instead of a DVE dacc add
                        # on the critical tail
                        pass
                    elif g == NG - 1 and dacc_started[p]:
                        # last group: per-exp half-width adds so the final
                        # add on the epilogue's critical chain is short
                        half = daccs[p][:, par * w : (par + 1) * w]
                        nc.vector.tensor_add(out=half, in0=half, in1=us)
                    elif par == 1:
                        if not dacc_started[p]:
                            nc.vector.tensor_copy(out=daccs[p], in_=u)
                            dacc_started[p] = True
                        else:
                            nc.vector.tensor_add(out=daccs[p], in0=daccs[p], in1=u)
                    if pending is not None:
                        emit_pv(pending)
                    pending = (p, pr, c, first, last, po, us, finishing)
                if pending is not None:
                    emit_pv(pending)

            # Emission plan: minimal prologue (kT block 0 + the qT blocks the
            # first pair needs), then every remaining projection is a JIT job
            # spread into the att groups just ahead of its deadline, so
            # per-window PE work tracks ACT's exp load.
            if NG == 4 and QNB == 4:
                # prologue: v0 before q1 so it doesn't wait behind the
                # xt1-gated q1 (wv lands at ~5.7us, well before xt1's 8.9us)
                pe_warmup(4)
                emit_bias()  # bqv lands ~2us (ACT q behind table load);
                # the warmup spans that, so no PE stall here anymore
                proj_k(0)
                proj_q(0)
                emit_bias_v()
                proj_v(0)
                proj_q(1)
                H = lambda f, b, h: (lambda: f(b, half=h))  # noqa: E731
                # (iteration, job): every projection must be emitted before
                # the first iteration whose scores/PV reads it (PE is
                # in-order, so a late job would deadlock). Deadlines: kT/vn
                # block 2g+1 is first read at group g iteration 4 (chunk
                # 8g+4); qT blocks 2,3 at group 1 iteration 8 (pair1 start).
                # Placements track the x-block DMA arrivals (~3.2us apart)
                # so no job stalls on its xt transfer.
                jobs_by_g = {
                    0: [(0, H(proj_k, 1, 0)), (1, H(proj_k, 1, 1)),
                        (2, H(proj_v, 1, 0)), (3, H(proj_v, 1, 1)),
                        (4, H(proj_k, 2, 0)), (5, H(proj_k, 2, 1)),
                        (6, H(proj_v, 2, 0)), (7, H(proj_v, 2, 1))],
                    1: [(0, H(proj_k, 3, 0)), (1, H(proj_k, 3, 1)),
                        (2, H(proj_v, 3, 0)), (3, H(proj_v, 3, 1)),
                        (4, H(proj_q, 2, 0)), (5, H(proj_q, 2, 1)),
                        (6, H(proj_q, 3, 0)), (7, H(proj_q, 3, 1)),
                        (9, H(proj_k, 4, 0)), (10, H(proj_k, 4, 1)),
                        (11, H(proj_v, 4, 0)), (12, H(proj_v, 4, 1)),
                        (14, H(proj_k, 5, 0)), (15, H(proj_k, 5, 1)),
                        (17, H(proj_v, 5, 0)), (18, H(proj_v, 5, 1))],
                    2: [(1, H(proj_k, 6, 0)), (2, H(proj_k, 6, 1)),
                        (4, H(proj_v, 6, 0)), (5, H(proj_v, 6, 1))],
                    3: [(0, H(proj_k, 7, 0)), (1, H(proj_k, 7, 1)),
                        (2, H(proj_v, 7, 0)), (3, H(proj_v, 7, 1))],
                }
            else:
                pe_warmup(4)
                emit_bias()
                emit_bias_v()
                for b in range(NBLK):
                    proj_block(b)
                jobs_by_g = {}
            att_all(jobs_by_g)

        if repeat > 1:
            loop_cm.__exit__(None, None, None)

    return nc


_NC_CACHE: dict = {}


def _get_nc(S: int = S_FULL, SQ: int = S_FULL // 2, repeat: int = 1):
    key = (S, SQ, repeat)
    if key not in _NC_CACHE:
        nc = bacc.Bacc("TRN2", debug=False)
        build_attention(nc, S, SQ, repeat)
        nc.compile()  # splits multi-waits into event semaphores (HW limit)
        _NC_CACHE[key] = nc
    return _NC_CACHE[key]


def _w_layout(W, bf16):
    # [D, 128] -> [128 part, DC*128]: w[p, c*128+n] = W[c*128+p, n]
    return np.ascontiguousarray(
        np.asarray(W, np.float32).reshape(8, P, D_QK).transpose(1, 0, 2).reshape(P, -1)
    ).astype(bf16)


def make_in_maps(x, Wq, bq, Wk, bk, Wv, bv):
    """Per-core input dicts. Core c = (batch c//2, query-half c%2).

    All tensors are pre-arranged into the kernel's SBUF layouts so on-device
    DMAs are contiguous (descriptor-count-bound otherwise). bk is
    mathematically irrelevant (it adds a per-query constant to that query's
    scores; softmax is shift-invariant), so it is dropped.
    """
    import ml_dtypes

    bf16 = ml_dtypes.bfloat16
    x = np.asarray(x, dtype=np.float32)
    NBLK = S_FULL // 512
    common = {
        "Wq": _w_layout(Wq, bf16),
        "Wk": _w_layout(Wk, bf16),
        "Wv": _w_layout(Wv, bf16),
        "bqv": np.ascontiguousarray(
            np.stack([np.asarray(bq), np.asarray(bv)]).astype(np.float32)
        ),
    }
    in_maps = []
    for c in range(N_CORES):
        b, h = divmod(c, 2)
        xb = x[b]  # [S, D]
        half = S_FULL // 2
        if h == 0:
            perm = xb
        else:
            perm = np.concatenate([xb[half:], xb[:half]], axis=0)
        # xh[b*128+p, c*512+s] = perm[b*512+s, c*128+p]
        xhf = (
            perm.reshape(NBLK, 512, 8, P)
            .transpose(0, 3, 2, 1)
            .reshape(NBLK * P, 8 * 512)
        )
        in_maps.append({"xh": np.ascontiguousarray(xhf).astype(bf16), **common})
    return in_maps


def assemble_output(results):
    """results: list of 8 per-core dicts with 'outT' [128, 2048]."""
    half = S_FULL // 2
    out = np.empty((B, S_FULL, D_QK), dtype=np.float32)
    for c in range(N_CORES):
        b, h = divmod(c, 2)
        out[b, h * half : (h + 1) * half, :] = results[c]["outT"].T
    return out


def kernel(x, Wq, bq, Wk, bk, Wv, bv):
    from concourse.bass_utils import run_bass_kernel_spmd

    nc = _get_nc()
    in_maps = make_in_maps(x, Wq, bq, Wk, bk, Wv, bv)
    res = run_bass_kernel_spmd(nc, in_maps, list(range(N_CORES)))
    return assemble_output(res.results)

